# revision 40
# baseline (speedup 1.0000x reference)
"""Trainium2 Bass kernel for nn_Attention_89902255440825.

Single-layer attention block: QKV proj + per-head RMS("mult" variant) +
RoPE + GQA causal attention with softmax(scores * sqrt(HD)) + O proj.

Sharding (8 NeuronCores, tensor-parallel over heads):
  core c: q heads {2c, 2c+1}  (wq cols 256c:256c+256)
          kv head c//2        (wk/wv cols 128*(c//2):...)
          wo rows 256c:256c+256  -> partial [S,H] outputs, summed on host.

v3 design (all primitives validated on HW):
  - Projections + scores in fp32r: single-pass matmuls, 1 cyc/row in the
    cost model for moving dim >= 256, measured HW rel-err 1.3e-4.
  - Token-major fused QKV projection: one [128m x 512f] PSUM bank per
    token block (q0|q1|k|v columns), so RMS sums, sigma and the k scale
    are plain per-partition ops -- no cross-partition broadcasts.
  - sigma = exp(0.5*ln(sum sq)) on Act; the activation-table list is
    pinned to natural_log_exp_and_others during compile so Square/Ln/
    Exp/Copy share one table (the greedy chooser thrashes otherwise).
  - RoPE (column-shifted muls) runs on the otherwise-idle Pool engine.
  - q/k transposed to feature-major via PE; V needs no transpose at all.
  - Softmax from PSUM: causal mask added by a constant fp16 matmul,
    row max on DVE, exp on Act -> fp16; q's RMS scale is applied inside
    exp via the per-partition scale AP (exact, fp32).
  - PV accumulates a ones-column of V for the denominator.
  - Separate PSUM tags for projections vs scores + interleaved emission
    (attention rows between projection blocks) keep the PE queue fed.
"""
import numpy as np
from contextlib import ExitStack

import concourse.bass as bass
import concourse.tile as tile
from concourse import bacc, mybir, bass_utils
from concourse.masks import make_identity

S = 2048
H = 2048
HD = 128
NH = 16
NKV = 4
NCORES = 8
HPC = NH // NCORES          # q heads per core = 2
FQ = HPC * HD               # q features per core = 256
NEGM = -60000.0             # fp16-representable mask additive
F32 = mybir.dt.float32
F32R = mybir.dt.float32r
F16 = mybir.dt.float16
AX = mybir.AxisListType.X
OP = mybir.AluOpType
ACTF = mybir.ActivationFunctionType

NKB = H // 128              # 16 contraction k-blocks
NMB = S // 128              # 16 token blocks
NQ = 4                      # quarters
VW = 129                    # v width incl. ones column
FALL = FQ + 2 * HD          # 512: q0|q1|k|v fused projection width

_prog_cache = {}


def _build():
    nc = bacc.Bacc("TRN2", target_bir_lowering=False, debug=False,
                   num_devices=NCORES)

    def din(name, shape, dt):
        return nc.dram_tensor(name, shape, dt, kind="ExternalInput").ap()

    xt_d = din("xt", [H, S], F32R)               # xT fp32
    w_d = din("w", [128, NKB, FALL], F32R)       # packed fused qkv weights
    wo_d = din("wo", [128, HPC, H], F16)
    cosq_d = din("cosq", [128, NMB, HD], F32)    # token-major rope tables
    sinq_d = din("sinq", [128, NMB, HD], F32)
    cosk_d = din("cosk", [128, NMB, HD], F32)
    sink_d = din("sink", [128, NMB, HD], F32)
    b_d = din("b", [1, FALL], F32R)              # fused bias row
    ones_d = din("ones", [1, 512], F32R)
    out_d = nc.dram_tensor("out", [S, H], F16, kind="ExternalOutput").ap()

    xt_r = xt_d.rearrange("(g p) m -> p g m", p=128)   # [128, 16, 2048]

    with tile.TileContext(nc) as tc, ExitStack() as ctx:
        const = ctx.enter_context(tc.tile_pool(name="const", bufs=1))
        wpool = ctx.enter_context(tc.tile_pool(name="wpool", bufs=1))
        big = ctx.enter_context(tc.tile_pool(name="big", bufs=1))
        xpool = ctx.enter_context(tc.tile_pool(name="xpool", bufs=2))
        btmp = ctx.enter_context(tc.tile_pool(name="btmp", bufs=2))
        cpool = ctx.enter_context(tc.tile_pool(name="cpool", bufs=2))
        psum = ctx.enter_context(tc.tile_pool(name="psum", bufs=1, space="PSUM"))

        # ---- constants ----
        ident16 = const.tile([128, 128], F16)
        make_identity(nc, ident16[:])
        ident32 = const.tile([128, 128], F32)
        make_identity(nc, ident32[:])
        cmask = const.tile([128, 4, 512], F16)
        for r in range(4):
            nc.vector.memset(cmask[:, r, :], 0.0)
            nc.gpsimd.affine_select(
                out=cmask[:, r, :], in_=cmask[:, r, :],
                compare_op=OP.is_ge, fill=NEGM,
                base=128 * r, channel_multiplier=1, pattern=[[-1, 512]],
            )

        # ---- weights / tables ----
        b_sb = wpool.tile([1, FALL], F32R, name="b_sb", tag="b_sb")
        nc.sync.dma_start(b_sb[:], b_d)
        ones_sb = wpool.tile([1, 512], F32R, name="ones_sb", tag="ones_sb")
        nc.sync.dma_start(ones_sb[:], ones_d)
        w_sb = wpool.tile([128, NKB, FALL], F32R, name="w_sb", tag="w_sb")
        cosq_sb = wpool.tile([128, NMB, HD], F32, name="cosq_sb", tag="cosq_sb")
        sinq_sb = wpool.tile([128, NMB, HD], F32, name="sinq_sb", tag="sinq_sb")
        cosk_sb = wpool.tile([128, NMB, HD], F32, name="cosk_sb", tag="cosk_sb")
        sink_sb = wpool.tile([128, NMB, HD], F32, name="sink_sb", tag="sink_sb")
        woh_sb = wpool.tile([128, HPC, H], F16, name="woh_sb", tag="woh_sb")

        # ---- persistent activations ----
        qh = big.tile([128, HPC, S], F32R)       # roped q, feature-major
        kh = big.tile([128, S], F32R)            # roped+scaled k, feature-major
        zrow = const.tile([128, 512], F32)
        nc.vector.memset(zrow[:], 0.0)
        for z in range(4):
            # rows read kh up to the 512-aligned chunk end before those
            # blocks are projected; zeros keep masked columns finite
            nc.vector.tensor_copy(kh[:, 512 * z:512 * z + 512], zrow[:])
        v_sb = big.tile([128, NMB, VW], F16)     # v token-major + ones col
        nc.vector.memset(v_sb[:, :, 128:129], 1.0)
        sig_all = big.tile([128, NMB, 4], F32)   # sigma per block: q0,q1,k

        xtiles = {}

        def proj_block(mb):
            """Fused QKV projection for token block mb + RMS/rope/transpose.
            Generator: yields at pipeline boundaries for emission weaving."""
            pj = psum.tile([128, FALL], F32, tag="csc", bufs=4,
                           name=f"pj{mb}")
            xt_t = xtiles[mb]
            for kb in range(8):
                nc.tensor.matmul(pj[:], xt_t[:, kb, :],
                                 w_sb[:, kb], start=(kb == 0), stop=False)
            yield
            for kb in range(8, NKB):
                nc.tensor.matmul(pj[:], xt_t[:, kb, :],
                                 w_sb[:, kb], start=False, stop=False)
            nc.tensor.matmul(pj[:], ones_sb[0:1, 0:128], b_sb[:],
                             start=False, stop=True)
            yield
            # copy q|k to SBUF first: releases the psum slot early and
            # unblocks rope; sigma reads the SBUF copy
            qk = btmp.tile([128, FQ + HD], F32, tag="qk", name=f"qk{mb}")
            nc.scalar.copy(qk[:], pj[:, 0:FQ + HD])
            nc.scalar.copy(v_sb[:, mb, 0:128], pj[:, FQ + HD:FALL])
            sqd = btmp.tile([128, 128], F16, tag="sqd", name=f"sqd{mb}")
            ssum = btmp.tile([128, 4], F32, tag="ssum", name=f"ssum{mb}")
            for c in range(3):
                nc.scalar.activation(sqd[:], qk[:, 128 * c:128 * c + 128],
                                     ACTF.Square, accum_out=ssum[:, c:c + 1])
            lnv = btmp.tile([128, 4], F32, tag="lnv", name=f"lnv{mb}")
            nc.scalar.activation(lnv[:, 0:3], ssum[:, 0:3], ACTF.Ln)
            nc.scalar.activation(sig_all[:, mb, 0:3], lnv[:, 0:3], ACTF.Exp,
                                 scale=0.5)
            yield
            # rope on the Pool engine (column-shifted muls)
            qr = btmp.tile([128, FQ], F32, tag="qr", name=f"qr{mb}")
            kr = btmp.tile([128, HD], F32, tag="kr", name=f"kr{mb}")
            cq = cosq_sb[:, mb]
            sq_ = sinq_sb[:, mb]
            for h in range(HPC):
                hs = 128 * h
                nc.gpsimd.tensor_mul(qr[:, hs:hs + 128], qk[:, hs:hs + 128],
                                     cq)
            rt = btmp.tile([128, HPC, HD], F32, tag="rt", name=f"rt{mb}")
            qk3 = qk[:, 0:FQ].rearrange("p (h d) -> p h d", h=HPC)
            qr3 = qr[:, 0:FQ].rearrange("p (h d) -> p h d", h=HPC)
            for h in range(HPC):
                nc.gpsimd.tensor_mul(rt[:, h, 0:64], qk3[:, h, 64:128],
                                     sq_[:, 0:64])
                nc.gpsimd.tensor_mul(rt[:, h, 64:128], qk3[:, h, 0:64],
                                     sq_[:, 64:128])
            nc.gpsimd.tensor_add(qr3[:], qr3[:], rt[:])
            ck = cosk_sb[:, mb]
            sk_ = sink_sb[:, mb]
            kq = qk[:, FQ:FQ + HD]
            nc.gpsimd.tensor_mul(kr[:], kq, ck)
            ktt = btmp.tile([128, HD], F32, tag="ktt", name=f"ktt{mb}")
            nc.gpsimd.tensor_mul(ktt[:, 0:64], kq[:, 64:128], sk_[:, 0:64])
            nc.gpsimd.tensor_mul(ktt[:, 64:128], kq[:, 0:64], sk_[:, 64:128])
            nc.gpsimd.tensor_add(kr[:], kr[:], ktt[:])
            nc.gpsimd.tensor_scalar_mul(kr[:], kr[:], sig_all[:, mb, 2:3])
            yield
            # transpose to feature-major fp32r
            for h in range(HPC):
                ptq = psum.tile([128, 128], F32, tag="t128", bufs=2,
                                name=f"ptq{mb}_{h}")
                nc.tensor.transpose(ptq[:], qr[:, 128 * h:128 * h + 128],
                                    ident32[:])
                nc.vector.tensor_copy(qh[:, h, 128 * mb:128 * mb + 128],
                                      ptq[:])
            ptk = psum.tile([128, 128], F32, tag="t128", bufs=2,
                            name=f"ptk{mb}")
            nc.tensor.transpose(ptk[:], kr[:], ident32[:])
            nc.vector.tensor_copy(kh[:, 128 * mb:128 * mb + 128], ptk[:])

        def attn_row(i):
            """Causal attention for token block i (both heads) + O proj."""
            r = i % 4
            nfull = i // 4
            nch = nfull + 1
            w = max(256, (r + 1) * 128)
            attn16 = cpool.tile([128, HPC, 128], F16, tag="attn16",
                                name=f"attn16_{i}")
            out_ps_h = {}
            chunks_by_h = {}
            pmx_by_h = {}
            for h in range(HPC):
                out_ps_h[h] = psum.tile([128, 132], F32, tag="tout", bufs=2,
                                        name=f"out_ps{i}_{h}")
                qblk = qh[:, h, 128 * i:128 * i + 128]
                pmx = cpool.tile([128, 8], F32, tag="pmx", bufs=3, name=f"pmx{i}_{h}")
                pss_chunks = []
                for c in range(nch):
                    wd = 512 if c < nfull else w
                    if c == nfull and nch == 4:
                        ps = psum.tile([128, 512], F32, tag="tout", bufs=2,
                                       name=f"ps{i}_{h}_{c}")
                    else:
                        ps = psum.tile([128, 512], F32, tag="csc", bufs=4,
                                       name=f"ps{i}_{h}_{c}")
                    nc.tensor.matmul(ps[:, 0:wd], qblk,
                                     kh[:, 512 * c:512 * c + wd],
                                     start=True, stop=(c < nfull))
                    if c == nfull:
                        nc.tensor.matmul(ps[:, 0:wd], ident16[:],
                                         cmask[:, r, 0:wd],
                                         start=False, stop=True)
                    nc.vector.reduce_max(pmx[:, c:c + 1], ps[:, 0:wd],
                                         axis=AX)
                    pss_chunks.append(ps)
                chunks_by_h[h] = pss_chunks
                pmx_by_h[h] = pmx
                yield
            for h in range(HPC):
                sg = sig_all[:, i, h:h + 1]
                pss_chunks = chunks_by_h[h]
                nm = cpool.tile([128, 1], F32, tag="nm", bufs=3, name=f"nm{i}_{h}")
                nc.vector.reduce_max(nm[:], pmx_by_h[h][:, 0:nch], axis=AX,
                                     negate=True)
                bcol = cpool.tile([128, 1], F32, tag="bcol", bufs=3,
                                  name=f"bcol{i}_{h}")
                nc.vector.tensor_mul(bcol[:], nm[:], sg)
                for c in range(nch):
                    wd = 512 if c < nfull else w
                    nb = 4 if c < nfull else r + 1
                    ps = pss_chunks[c]
                    p16 = cpool.tile([128, 512], F16, tag="p16", bufs=3,
                                     name=f"p16_{i}_{h}_{c}")
                    with tc.high_priority(offset=800):
                        nc.scalar.activation(p16[:, 0:wd], ps[:, 0:wd],
                                             ACTF.Exp, bias=bcol[:],
                                             scale=sg)
                    pst = psum.tile([128, 4, 128], F16, tag="t128",
                                    bufs=2, name=f"pst{i}_{h}_{c}")
                    for b in range(nb):
                        nc.tensor.transpose(
                            pst[:, b], p16[:, 128 * b:128 * b + 128],
                            ident16[:])
                    pt = cpool.tile([128, 4, 128], F16, tag="pt", bufs=3,
                                    name=f"pt{i}_{h}_{c}")
                    nc.vector.tensor_copy(pt[:, 0:nb], pst[:, 0:nb])
                    for b in range(nb):
                        nkb = 4 * c + b
                        nc.tensor.matmul(out_ps_h[h][:, 0:VW], pt[:, b],
                                         v_sb[:, nkb, 0:VW],
                                         start=(nkb == 0),
                                         stop=(nkb == i))
                linv = cpool.tile([128, 1], F32, tag="linv",
                                  name=f"linv{i}_{h}")
                nc.vector.reciprocal(linv[:], out_ps_h[h][:, 128:129])
                at = cpool.tile([128, 128], F16, tag="at", name=f"at{i}_{h}")
                nc.vector.tensor_scalar_mul(at[:], out_ps_h[h][:, 0:128],
                                            linv[:])
                pat = psum.tile([128, 128], F16, tag="t128", bufs=2,
                                name=f"pat{i}_{h}")
                nc.tensor.transpose(pat[:], at[:], ident16[:])
                nc.vector.tensor_copy(attn16[:, h], pat[:])
                yield
            for nh_ in range(4):
                ns = slice(512 * nh_, 512 * nh_ + 512)
                po = psum.tile([128, 512], F32, tag="tout", bufs=2,
                               name=f"po{i}_{nh_}")
                nc.tensor.matmul(po[:], attn16[:, 0], woh_sb[:, 0, ns],
                                 start=True, stop=False)
                nc.tensor.matmul(po[:], attn16[:, 1], woh_sb[:, 1, ns],
                                 start=False, stop=True)
                ob = cpool.tile([128, 512], F16, tag="ob", bufs=3,
                                name=f"ob{i}_{nh_}")
                nc.scalar.copy(ob[:], po[:])
                nc.sync.dma_start(out_d[128 * i:128 * i + 128, ns], ob[:])

        # ================= main interleaved schedule =================
        def load_block(mb):
            ms = slice(128 * mb, 128 * mb + 128)
            xt_t = xpool.tile([128, NKB, 128], F32R, tag="xt", bufs=5,
                              name=f"xtb{mb}")
            nc.sync.dma_start(xt_t[:], xt_r[:, :, ms])
            xtiles[mb] = xt_t
            if mb % 4 == 0:
                qsl = slice(mb, mb + 4)
                for tbl, dram in ((cosk_sb, cosk_d), (sink_sb, sink_d),
                                  (cosq_sb, cosq_d), (sinq_sb, sinq_d)):
                    nc.sync.dma_start(tbl[:, qsl], dram[:, qsl])

        def weave(*gens):
            gens = [g for g in gens if g is not None]
            while gens:
                nxt = []
                for g in gens:
                    try:
                        next(g)
                        nxt.append(g)
                    except StopIteration:
                        pass
                gens = nxt

        def weave2(pgen, agen):
            # P1 A1 P2 A2 A3 A4 P3 A5 P4 P5: both heads' softmax (A3/A4)
            # hit the Act queue before the next proj's sigma chain (P3)
            for g in (pgen, agen, pgen, agen, agen, agen, pgen, agen,
                      pgen, pgen):
                try:
                    next(g)
                except StopIteration:
                    pass
            weave(pgen, agen)

        # interleave w chunks with block-0 x so kb matmuls start early
        xt_t0 = xpool.tile([128, NKB, 128], F32R, tag="xt", bufs=5,
                           name="xtb0")
        xtiles[0] = xt_t0
        for g in range(8):
            ksl = slice(2 * g, 2 * g + 2)
            nc.sync.dma_start(w_sb[:, ksl], w_d[:, ksl])
            nc.sync.dma_start(xt_t0[:, ksl], xt_r[:, ksl, 0:128])
        for tbl, dram in ((cosk_sb, cosk_d), (sink_sb, sink_d),
                          (cosq_sb, cosq_d), (sinq_sb, sinq_d)):
            nc.sync.dma_start(tbl[:, 0:4], dram[:, 0:4])
        load_block(1)
        nc.sync.dma_start(woh_sb[:], wo_d)
        load_block(2)
        load_block(3)
        weave(proj_block(0))
        for mb in range(1, NMB):
            if mb + 3 < NMB:
                load_block(mb + 3)
            weave2(proj_block(mb), attn_row(mb - 1))
        weave(attn_row(NMB - 1))

    # pin the activation-table choice to the one table that holds
    # Square/Ln/Exp/Copy so the load-insertion pass emits a single load
    import concourse.bacc as bacc_mod
    orig = bacc_mod.get_activation_tables

    def pinned(arch):
        t = orig(arch)
        keep = "natural_log_exp_and_others"
        return {name: (funcs if name == keep else set())
                for name, funcs in t.items()}

    bacc_mod.get_activation_tables = pinned
    try:
        nc.compile()
    finally:
        bacc_mod.get_activation_tables = orig
    return nc


def kernel(**inputs):
    x = np.asarray(inputs["x"], np.float32)
    cos = np.asarray(inputs["cos"], np.float32)
    sin = np.asarray(inputs["sin"], np.float32)
    am = np.asarray(inputs["attention_mask"]).reshape(S, S).astype(bool)
    wq = np.asarray(inputs["wq"], np.float32)
    bq = np.asarray(inputs["bq"], np.float32)
    wk = np.asarray(inputs["wk"], np.float32)
    bk = np.asarray(inputs["bk"], np.float32)
    wv = np.asarray(inputs["wv"], np.float32)
    bv = np.asarray(inputs["bv"], np.float32)
    wo = np.asarray(inputs["wo"], np.float32)
    bo = np.asarray(inputs["bo"], np.float32)
    qn = np.asarray(inputs["q_norm_w"], np.float32)
    kn = np.asarray(inputs["k_norm_w"], np.float32)

    assert x.shape == (1, S, H)
    assert (am == np.triu(np.ones((S, S), dtype=bool), k=1)).all(), \
        "kernel supports the causal mask only"

    if "p" not in _prog_cache:
        _prog_cache["p"] = _build()
        _prog_cache[True] = _prog_cache["p"]  # legacy key for test.py
    nc = _prog_cache["p"]

    xT = np.ascontiguousarray(x[0].T)
    rolled_q = np.roll(qn, -64)     # rot(q*qn)[i] = rot(q)[i] * qn[(i+64)%128]
    rolled_k = np.roll(kn, -64)
    halfsign = np.where(np.arange(HD) < 64, np.float32(-1.0), np.float32(1.0))
    # token-major rope tables [m, d] packed to [p, mb, d]; the k tables
    # absorb 1/sqrt(HD) (reference k-RMS uses mean; q side supplies the
    # softmax sqrt(HD) via sigma_q = sqrt(sum q^2))
    ksc = np.float32(1.0 / np.sqrt(HD))

    def packm(t):   # [S, HD] -> [128, NMB, HD]
        return np.ascontiguousarray(
            t.reshape(NMB, 128, HD).transpose(1, 0, 2).astype(np.float32))

    cosq = packm(cos * qn[None, :])
    sinq = packm(sin * (rolled_q * halfsign)[None, :])
    cosk = packm(cos * kn[None, :] * ksc)
    sink = packm(sin * (rolled_k * halfsign)[None, :] * ksc)
    ones = np.ones((1, 512), np.float32)

    in_maps = []
    for c in range(NCORES):
        fq = slice(c * FQ, (c + 1) * FQ)
        g = c // 2
        fk = slice(g * HD, (g + 1) * HD)
        wall = np.concatenate([wq[:, fq], wk[:, fk], wv[:, fk]], axis=1)
        ball = np.concatenate([bq[fq], bk[fk], bv[fk]]).reshape(1, FALL)
        m = dict(
            xt=xT,
            w=np.ascontiguousarray(
                wall.reshape(NKB, 128, FALL).transpose(1, 0, 2)),
            wo=np.ascontiguousarray(
                wo[fq, :].astype(np.float16).reshape(HPC, 128, H)
                .transpose(1, 0, 2)),
            cosq=cosq, sinq=sinq, cosk=cosk, sink=sink,
            b=ball.astype(np.float32), ones=ones,
        )
        in_maps.append(m)

    res = bass_utils.run_bass_kernel_spmd(nc, in_maps,
                                          core_ids=list(range(NCORES)))
    acc = np.zeros((S, H), np.float64)
    for c in range(NCORES):
        acc += res.results[c]["out"].astype(np.float64)
    out = (acc + bo[None, :]).astype(np.float32)
    return out.reshape(1, S, H)
